# revision 4
# baseline (speedup 1.0000x reference)
"""DenseGATConv Bass/Tile kernel for Trainium2, SPMD over 8 NeuronCores.

Problem: B=4, N=2048, F=128, H=4, C=64 dense GAT layer.
  xh = (x @ W).reshape(B,N,H,C)
  a_src[b,j,h] = xh . att_src ; a_dst[b,i,h] = xh . att_dst
  s[b,i,j,h]  = a_src[j] + a_dst[i]
  alpha = softmax_j( mask(adj, leaky_relu(s, 0.2)) )
  out[b,i,:]  = concat_h( sum_j alpha * xh[b,j,h,:] ) + bias

Key algebraic transform (avoids any exp over the N*N*H grid):
  exp(lrelu(s)) = exp(s) * max(exp(-0.8 s), 1)
               = [exp(a_src_j) * exp(a_dst_i)] * max(Q'_i * R'_j, 1)
  with Q' = exp(-0.8 a_dst), R' = exp(-0.8 a_src).
  exp(a_dst_i) cancels in the softmax ratio; exp(a_src_j) folds into
  xh1 = exp(a_src) * xh.  The masked grid weight becomes
     G[j,i] = adj[i,j] * max(Q'_i R'_j, 1)
  computed with 2 DVE ops per tile:
     T = (Q'_bcast max invR'_j) * R'_j        (tensor_scalar, dual op)
     G = T * adjT                             (tensor_tensor)
  then num/den via PE matmuls:  acc[c,i] (+)= xh1[j,c]^T @ G[j,i]
  where xh1 carries an extra column exp(a_src) so row 64 of acc is den.

Sharding: core = b*2 + ihalf. Each core computes 1024 destination rows of
one batch and reads the full source side of that batch.
"""

import numpy as np

import concourse.bacc as bacc
import concourse.bass as bass
import concourse.tile as tile
from concourse import mybir
from concourse.bass_utils import run_bass_kernel_spmd
from concourse.masks import make_identity

B, N, F = 4, 2048, 128
H, C = 4, 64
HC = H * C
NEG_SLOPE = 0.2
N_CORES = 8
ID = N // 2          # dest rows per core
NT = N // 128        # 16 source tiles
NKD = ID // 512      # 2 dest 512-chunks
F32 = mybir.dt.float32
F16 = mybir.dt.float16

_NC_CACHE = {}


def build_nc(reps: int = 1):
    nc = bacc.Bacc("TRN2", target_bir_lowering=False, debug=False, num_devices=1)

    d_xT = nc.dram_tensor("xT", [F, N], F32, kind="ExternalInput").ap()
    d_xTd = nc.dram_tensor("xTd", [F, ID], F32, kind="ExternalInput").ap()
    d_adjT = nc.dram_tensor("adjT", [NT, 128, ID], F16, kind="ExternalInput").ap()
    d_wcat = nc.dram_tensor("Wcat", [F, HC + 12], F32, kind="ExternalInput").ap()
    d_wadst = nc.dram_tensor("Wadst", [F, H], F32, kind="ExternalInput").ap()
    d_bias = nc.dram_tensor("biasv", [1, HC], F32, kind="ExternalInput").ap()
    d_out = nc.dram_tensor("out", [ID, HC], F32, kind="ExternalOutput").ap()

    EXP = mybir.ActivationFunctionType.Exp
    CPY = mybir.ActivationFunctionType.Copy

    with tile.TileContext(nc) as tc:
        with tc.tile_pool(name="const", bufs=1) as const:
            ident = const.tile([128, 128], F32)
            make_identity(nc, ident)
            ones1 = const.tile([1, 128], F32)
            nc.vector.memset(ones1, 1.0)

            xT = const.tile([F, N], F32)
            nc.sync.dma_start(out=xT, in_=d_xT)
            xTd = const.tile([F, ID], F32)
            nc.sync.dma_start(out=xTd, in_=d_xTd)
            wcat = const.tile([F, HC + 12], F32)
            nc.sync.dma_start(out=wcat, in_=d_wcat)
            wadst = const.tile([F, H], F32)
            nc.sync.dma_start(out=wadst, in_=d_wadst)
            bias_sb = const.tile([1, HC], F32)
            nc.sync.dma_start(out=bias_sb, in_=d_bias)

            # persistent per-core tensors
            xh1 = const.tile([128, NT, H, 65], F16)     # [F1*xh | F1] per (t,h)
            expv = const.tile([128, NT, 12], F32)       # F1 | R' | invR' per t
            q_bc = const.tile([128, H, ID], F16)        # Q' broadcast per head
            bias_bc = const.tile([128, HC], F32)

            # ---------------- phase A: projections ----------------
            with tc.tile_pool(name="psA", bufs=2, space="PSUM") as psA, \
                 tc.tile_pool(name="psD", bufs=2, space="PSUM") as psDp, \
                 tc.tile_pool(name="psB", bufs=2, space="PSUM") as psBp:
                for t in range(NT):
                    ps = psA.tile([128, HC + 12], F32)
                    nc.tensor.matmul(ps, xT[:, t * 128:(t + 1) * 128], wcat,
                                     start=True, stop=True)
                    # one exp over the 12 pre-scaled projection cols
                    nc.scalar.activation(expv[:, t, :], ps[:, HC:HC + 12], EXP)
                    # xh1[:, t, h, 0:64] = F1_h * xh_h   (ACT copy, per-part scale)
                    for h in range(H):
                        nc.scalar.activation(
                            xh1[:, t, h, 0:64], ps[:, h * 64:(h + 1) * 64], CPY,
                            scale=expv[:, t, h:h + 1])
                    # xh1[:, t, h, 64] = F1_h  (strided copy, all 4 heads at once)
                    nc.scalar.activation(xh1[:, t, :, 64:65], expv[:, t, 0:4], CPY)

                # a_dstT rows (pre-scaled by -0.8 on host), one head at a time so
                # every PSUM access stays partition-0 based
                qrow = const.tile([1, H, ID], F32)
                for h in range(H):
                    for k in range(NKD):
                        psd = psDp.tile([1, 512], F32)
                        nc.tensor.matmul(psd, wadst[:, h:h + 1],
                                         xTd[:, k * 512:(k + 1) * 512],
                                         start=True, stop=True)
                        nc.scalar.activation(
                            qrow[0:1, h, k * 512:(k + 1) * 512], psd, EXP)
                # broadcast Q' rows to 128 partitions via ones outer product
                for h in range(H):
                    for k in range(NKD):
                        psb = psBp.tile([128, 512], F32)
                        nc.tensor.matmul(psb, ones1,
                                         qrow[0:1, h, k * 512:(k + 1) * 512],
                                         start=True, stop=True)
                        nc.scalar.activation(
                            q_bc[:, h, k * 512:(k + 1) * 512], psb, CPY)
                # bias broadcast
                psb2 = psBp.tile([128, HC], F32, tag="psbias", bufs=1)
                nc.tensor.matmul(psb2, ones1, bias_sb, start=True, stop=True)
                nc.scalar.activation(bias_bc, psb2, CPY)

            # ---------------- phase B: grid + matmul accumulate ----------------
            with tc.tile_pool(name="ep_sb", bufs=1) as epsb:
                with tc.tile_pool(name="acc", bufs=1, space="PSUM") as accp:
                    acc = {}
                    for h in range(H):
                        for k in range(NKD):
                            acc_t = accp.tile([65, 512], F32, tag=f"acc{h}_{k}",
                                              name=f"acc{h}_{k}")
                            acc[(h, k)] = acc_t

                    with tc.tile_pool(name="adj", bufs=3) as adjp, \
                         tc.tile_pool(name="grid", bufs=4) as gridp:
                        for rep in range(reps):
                            for t in range(NT):
                                adjt = adjp.tile([128, ID], F16)
                                nc.sync.dma_start(out=adjt, in_=d_adjT[t])
                                for h in range(H):
                                    tt = gridp.tile([128, ID], F16, tag="T")
                                    nc.vector.tensor_scalar(
                                        out=tt, in0=q_bc[:, h, :],
                                        scalar1=expv[:, t, 8 + h:9 + h],   # invR'
                                        scalar2=expv[:, t, 4 + h:5 + h],   # R'
                                        op0=mybir.AluOpType.max,
                                        op1=mybir.AluOpType.mult)
                                    g = gridp.tile([128, ID], F16, tag="G")
                                    nc.vector.tensor_tensor(
                                        out=g, in0=tt, in1=adjt,
                                        op=mybir.AluOpType.mult)
                                    lhsT = xh1[:, t, h, :]
                                    first = (rep == 0 and t == 0)
                                    last = (rep == reps - 1 and t == NT - 1)
                                    for k in range(NKD):
                                        nc.tensor.matmul(
                                            acc[(h, k)], lhsT,
                                            g[:, k * 512:(k + 1) * 512],
                                            start=first, stop=last)

                    # evacuate accumulators to SBUF (ACT is close to PSUM)
                    s_tiles = {}
                    for h in range(H):
                        for k in range(NKD):
                            s = epsb.tile([65, 512], F32, tag=f"s{h}_{k}",
                                          name=f"s{h}_{k}")
                            nc.scalar.activation(s, acc[(h, k)], CPY)
                            s_tiles[(h, k)] = s

                # acc PSUM released here
                # ------------- phase C: transpose + divide + bias + out -------------
                with tc.tile_pool(name="ep_ps", bufs=4, space="PSUM") as epps, \
                     tc.tile_pool(name="ep_sm", bufs=4) as epsm, \
                     tc.tile_pool(name="outp", bufs=2) as outp:
                    for k in range(NKD):
                        osb = [outp.tile([128, HC], F32, tag=f"o{kk}", name=f"o{kk}")
                               for kk in range(4)]
                        for h in range(H):
                            pt = epps.tile([128, 4, 65], F32)
                            for kk in range(4):
                                nc.tensor.transpose(
                                    pt[:, kk, :],
                                    s_tiles[(h, k)][:, kk * 128:(kk + 1) * 128],
                                    ident[0:65, 0:65])
                            rec = epsm.tile([128, 4, 1], F32)
                            nc.vector.reciprocal(rec, pt[:, :, 64:65])
                            for kk in range(4):
                                nc.vector.scalar_tensor_tensor(
                                    out=osb[kk][:, h * 64:(h + 1) * 64],
                                    in0=pt[:, kk, 0:64],
                                    scalar=rec[:, kk, 0:1],
                                    in1=bias_bc[:, h * 64:(h + 1) * 64],
                                    op0=mybir.AluOpType.mult,
                                    op1=mybir.AluOpType.add)
                        for kk in range(4):
                            r0 = (k * 4 + kk) * 128
                            nc.sync.dma_start(out=d_out[r0:r0 + 128, :],
                                              in_=osb[kk])

    nc.compile()
    return nc


def _get_nc(reps: int = 1):
    if reps not in _NC_CACHE:
        _NC_CACHE[reps] = build_nc(reps)
    return _NC_CACHE[reps]


def make_in_maps(x, adj, W, att_src, att_dst, bias):
    x = np.asarray(x, dtype=np.float32)
    adj = np.asarray(adj, dtype=np.float32)
    W = np.asarray(W, dtype=np.float32)
    att_src = np.asarray(att_src, dtype=np.float32)
    att_dst = np.asarray(att_dst, dtype=np.float32)
    bias = np.asarray(bias, dtype=np.float32)

    # weight prep: fold per-head attention dots into projection columns
    wa_src = np.stack([W[:, h * C:(h + 1) * C] @ att_src[h] for h in range(H)], 1)
    wa_dst = np.stack([W[:, h * C:(h + 1) * C] @ att_dst[h] for h in range(H)], 1)
    wcat = np.concatenate([W, wa_src, -0.8 * wa_src, 0.8 * wa_src], axis=1)
    wcat = np.ascontiguousarray(wcat, dtype=np.float32)          # [F, 268]
    wadst = np.ascontiguousarray(-0.8 * wa_dst, dtype=np.float32)  # [F, 4]

    adjl = adj.copy()
    idx = np.arange(N)
    adjl[:, idx, idx] = 1.0

    in_maps = []
    for c in range(N_CORES):
        b, half = c // 2, c % 2
        xT = np.ascontiguousarray(x[b].T, dtype=np.float32)
        xTd = np.ascontiguousarray(x[b, half * ID:(half + 1) * ID, :].T,
                                   dtype=np.float32)
        adjT = np.ascontiguousarray(
            adjl[b].T[:, half * ID:(half + 1) * ID]).astype(np.float16)
        in_maps.append({
            "xT": xT,
            "xTd": xTd,
            "adjT": adjT.reshape(NT, 128, ID),
            "Wcat": wcat,
            "Wadst": wadst,
            "biasv": bias.reshape(1, HC),
        })
    return in_maps


def assemble(results):
    out = np.empty((B, N, HC), dtype=np.float32)
    for c in range(N_CORES):
        b, half = c // 2, c % 2
        out[b, half * ID:(half + 1) * ID, :] = results[c]["out"]
    return out


def kernel(x, adj, W, att_src, att_dst, bias):
    nc = _get_nc(1)
    in_maps = make_in_maps(x, adj, W, att_src, att_dst, bias)
    res = run_bass_kernel_spmd(nc, in_maps, list(range(N_CORES)))
    return assemble(res.results)
